# revision 4
# baseline (speedup 1.0000x reference)
"""GCN (3-layer) + frozen ResNet18 + concat head on 8 Trainium2 NeuronCores.

Sharding:
  - GCN: nodes partitioned by destination across the 8 cores (6250
    nodes/core); per-layer gather tables (bf16, 128-col padded rows)
    exchanged via AllGather.  Edge aggregation = dma_gather of source rows
    (edges pre-sorted by dest sub-block, split by source half for int16
    indexing) + one-hot matmul scatter-add into PSUM.
  - ResNet18: batch split 4 images/core, weights replicated (bf16),
    convs lowered to shifted matmuls (stem via host space-to-depth +
    DMA-built im2col); feature vectors AllGathered.
  - Head computed redundantly on every core.
"""

import numpy as np
import ml_dtypes

import concourse.bass as bass
import concourse.bacc as bacc
import concourse.mybir as mybir
import concourse.tile as tile
from concourse import bass_utils
from concourse.masks import make_identity

FP32 = mybir.dt.float32
BF16 = mybir.dt.bfloat16
I16 = mybir.dt.int16
AF = mybir.ActivationFunctionType
OP = mybir.AluOpType

NCORES = 8
N, E, F, B, NCLS = 50000, 800000, 64, 32, 10
NPB = N // NCORES            # 6250 real nodes per core
NSB = 49                     # sub-blocks of 128 dest nodes
NPBP = NSB * 128             # 6272 padded nodes per core
NTOT = NCORES * NPBP         # 50176 padded table rows
HALF = NTOT // 2             # 25088  (< 32768 -> int16 indexable)
TF = 128                     # table row: 64 real features + 64 zeros (bf16)
MAXCALL = 32                 # chunks per dma_gather call (4096 idx)

_BLOCK_CFG = [
    # (cin, cout, H_in, stride, has_ds)
    (64, 64, 56, 1, False), (64, 64, 56, 1, False),
    (64, 128, 56, 2, True), (128, 128, 28, 1, False),
    (128, 256, 28, 2, True), (256, 256, 14, 1, False),
    (256, 512, 14, 2, True), (512, 512, 7, 1, False),
]
_STAGE_OF_BLOCK = [0, 0, 1, 1, 2, 2, 3, 3]
TAPS9 = [(dh, dw) for dh in range(3) for dw in range(3)]


# ----------------------------------------------------------------- host prep

def _prep_edges(edge_index):
    src = np.concatenate([edge_index[0].astype(np.int64),
                          np.arange(N, dtype=np.int64)])
    dst = np.concatenate([edge_index[1].astype(np.int64),
                          np.arange(N, dtype=np.int64)])
    pad_src = (src // NPB) * NPBP + (src % NPB)
    dst_core = dst // NPB
    dloc = dst % NPB
    sb = dloc // 128
    dmod = dloc % 128
    half = pad_src // HALF
    rel = (pad_src % HALF).astype(np.int64)

    key = (dst_core * NSB + sb) * 2 + half
    order = np.argsort(key, kind="stable")
    key_s = key[order]
    rel_s = rel[order]
    dmod_s = dmod[order]
    bounds = np.searchsorted(key_s, np.arange(NCORES * NSB * 2 + 1))
    cnt = np.diff(bounds).reshape(NCORES, NSB, 2)

    ncht = np.ceil(cnt.max(axis=0) / 128).astype(np.int64)  # [NSB, 2]
    NCH = int(ncht.sum())

    groups = []
    s0 = 0
    while s0 < NSB:
        s1 = s0 + 1
        while s1 < NSB:
            a = ncht[s0:s1 + 1, 0].sum()
            b = ncht[s0:s1 + 1, 1].sum()
            if a > MAXCALL or b > MAXCALL:
                break
            s1 += 1
        groups.append((s0, s1))
        s0 = s1

    proc_off = np.zeros((NSB, 2), dtype=np.int64)
    t = 0
    for s in range(NSB):
        for h in range(2):
            proc_off[s, h] = t
            t += ncht[s, h]

    call_specs = []          # (half, [(sb, nch)...], chunk_col0)
    call_of_sb = {}
    c0 = 0
    for (s0, s1) in groups:
        for h in range(2):
            lst = [(s, int(ncht[s, h])) for s in range(s0, s1)]
            call_specs.append((h, lst, c0))
            off = 0
            for s, n in lst:
                call_of_sb[(s, h)] = (len(call_specs) - 1, off)
                off += n
            c0 += off
    assert c0 == NCH

    dstloc = np.full((NCORES, 128, NCH), -1.0, dtype=np.float32)
    idx16 = np.zeros((NCORES, 16, NCH * 8), dtype=np.int16)
    for c in range(NCORES):
        for s in range(NSB):
            for h in range(2):
                k = (c * NSB + s) * 2 + h
                i0, i1 = bounds[k], bounds[k + 1]
                n = i1 - i0
                nch = int(ncht[s, h])
                if nch == 0:
                    continue
                relv = np.zeros(nch * 128, dtype=np.int16)
                dmv = np.full(nch * 128, -1.0, dtype=np.float32)
                relv[:n] = rel_s[i0:i1]
                dmv[:n] = dmod_s[i0:i1]
                p0 = int(proc_off[s, h])
                dstloc[c, :, p0:p0 + nch] = dmv.reshape(nch, 128).T
                ci, off = call_of_sb[(s, h)]
                q0 = call_specs[ci][2] + off
                idx16[c, :, q0 * 8:(q0 + nch) * 8] = relv.reshape(nch * 8, 16).T
    idx16 = np.tile(idx16, (1, 8, 1))  # replicate across the 8 Q7 cores
    return dict(ncht=ncht, NCH=NCH, groups=groups, proc_off=proc_off,
                call_specs=call_specs, call_of_sb=call_of_sb,
                dstloc=dstloc, idx16=np.ascontiguousarray(idx16))


def _bf(x):
    return np.asarray(x, dtype=ml_dtypes.bfloat16)


def _conv_w(w):
    """[Cout, Cin, 3, 3] -> [Cin, 9*Cout] bf16 (tap-major columns)."""
    cout, cin = w.shape[0], w.shape[1]
    return _bf(np.ascontiguousarray(w.transpose(1, 2, 3, 0).reshape(cin, 9 * cout)))


def _prep_resnet(params):
    out = {}
    w1 = np.asarray(params["stem_w"], np.float32)  # [64,3,7,7]
    ws = np.zeros((96, 128), np.float32)           # [K rows, kt*64]
    for bi, bb in enumerate(range(-2, 2)):
        for ai, aa in enumerate(range(-2, 2)):
            t = bi * 4 + ai
            kt, row = divmod(t, 8)
            for c in range(3):
                for di in range(2):
                    for dj in range(2):
                        kh = 2 * bb + di + 3
                        kw = 2 * aa + dj + 3
                        if 0 <= kh < 7 and 0 <= kw < 7:
                            ws[row * 12 + c * 4 + di * 2 + dj,
                               kt * 64:(kt + 1) * 64] = w1[:, c, kh, kw]
    out["ws"] = _bf(ws)
    out["gs"] = np.asarray(params["stem_g"], np.float32).reshape(-1, 1)
    out["bs"] = np.asarray(params["stem_b"], np.float32).reshape(-1, 1)
    for i, blk in enumerate(params["blocks"]):
        out[f"w{i}a"] = _conv_w(np.asarray(blk["w1"], np.float32))
        out[f"w{i}b"] = _conv_w(np.asarray(blk["w2"], np.float32))
        out[f"g{i}a"] = np.asarray(blk["g1"], np.float32).reshape(-1, 1)
        out[f"b{i}a"] = np.asarray(blk["b1"], np.float32).reshape(-1, 1)
        out[f"g{i}b"] = np.asarray(blk["g2"], np.float32).reshape(-1, 1)
        out[f"b{i}b"] = np.asarray(blk["b2"], np.float32).reshape(-1, 1)
        if "wd" in blk:
            wd = np.asarray(blk["wd"], np.float32)[:, :, 0, 0]
            out[f"wd{i}"] = _bf(np.ascontiguousarray(wd.T))
            out[f"gd{i}"] = np.asarray(blk["gd"], np.float32).reshape(-1, 1)
            out[f"bd{i}"] = np.asarray(blk["bd"], np.float32).reshape(-1, 1)
    return out


def _s2d_pad(img):
    """[3,224,224] f32 -> [12,115,115] bf16 (space-to-depth 2, pad 2/1)."""
    s = img.reshape(3, 112, 2, 112, 2).transpose(0, 2, 4, 1, 3).reshape(12, 112, 112)
    o = np.zeros((12, 115, 115), np.float32)
    o[:, 2:114, 2:114] = s
    return _bf(o)


# ------------------------------------------------------------- device kernel

def build_kernel(meta, resw):
    ncht, groups, proc_off = meta["ncht"], meta["groups"], meta["proc_off"]
    NCH = meta["NCH"]
    call_specs, call_of_sb = meta["call_specs"], meta["call_of_sb"]

    nc = bacc.Bacc(num_devices=NCORES)

    din = {}
    din["xb"] = nc.dram_tensor("xb", [NPBP, F], FP32, kind="ExternalInput")
    din["idx16"] = nc.dram_tensor("idx16", [128, NCH * 8], I16, kind="ExternalInput")
    din["dstloc"] = nc.dram_tensor("dstloc", [128, NCH], FP32, kind="ExternalInput")
    din["batchv"] = nc.dram_tensor("batchv", [128, NSB], FP32, kind="ExternalInput")
    for w in ("W1", "W2", "W3"):
        din[w] = nc.dram_tensor(w, [F, F], FP32, kind="ExternalInput")
    for bn in ("b1r", "b2r", "b3r"):
        din[bn] = nc.dram_tensor(bn, [128, F], FP32, kind="ExternalInput")
    din["linW"] = nc.dram_tensor("linW", [576, NCLS], FP32, kind="ExternalInput")
    din["linb"] = nc.dram_tensor("linb", [1, NCLS], FP32, kind="ExternalInput")
    din["xs2d"] = nc.dram_tensor("xs2d", [4, 12, 115, 115], BF16, kind="ExternalInput")
    for k, v in resw.items():
        dt = BF16 if v.dtype == ml_dtypes.bfloat16 else FP32
        din[k] = nc.dram_tensor(k, list(v.shape), dt, kind="ExternalInput")
    out_d = nc.dram_tensor("out", [B, NCLS], FP32, kind="ExternalOutput")

    with tile.TileContext(nc, num_cores=NCORES) as tc:
        with (tc.tile_pool(name="persist", bufs=1) as pp,
              tc.tile_pool(name="dram", bufs=1, space="DRAM") as dram,
              tc.tile_pool(name="acts", bufs=2) as ap_,
              tc.tile_pool(name="msgs", bufs=2) as mp,
              tc.tile_pool(name="oh", bufs=4) as ohp,
              tc.tile_pool(name="h", bufs=3) as hp_,
              tc.tile_pool(name="fm", bufs=2) as fmp,
              tc.tile_pool(name="scratch", bufs=2) as sp_,
              tc.tile_pool(name="sps", bufs=2, space="PSUM") as sps,
              tc.tile_pool(name="tpp", bufs=1, space="PSUM") as tpp,
              tc.tile_pool(name="xwp", bufs=1, space="PSUM") as xwp,
              tc.tile_pool(name="cpsp", bufs=2, space="PSUM") as cpsp,
              tc.tile_pool(name="plp", bufs=1, space="PSUM") as plp,
              tc.tile_pool(name="cnp", bufs=1, space="PSUM") as cnp):

            # ------------- persistent loads / constants
            src16 = pp.tile([128, NCH * 8], I16, name="src16")
            nc.sync.dma_start(src16[:], din["idx16"][:])
            dstloc = pp.tile([128, NCH], FP32, name="dstloc_sb")
            nc.sync.dma_start(dstloc[:], din["dstloc"][:])
            batchv = pp.tile([128, NSB], FP32, name="batchv_sb")
            nc.sync.dma_start(batchv[:], din["batchv"][:])
            iota128 = pp.tile([128, 128], FP32, name="iota128")
            nc.gpsimd.iota(iota128[:], pattern=[[1, 128]], base=0,
                           channel_multiplier=0,
                           allow_small_or_imprecise_dtypes=True)
            iota32 = pp.tile([128, 32], FP32, name="iota32")
            nc.gpsimd.iota(iota32[:], pattern=[[1, 32]], base=0,
                           channel_multiplier=0,
                           allow_small_or_imprecise_dtypes=True)
            ident = pp.tile([128, 128], FP32, name="ident")
            make_identity(nc, ident[:])
            ones_bf = pp.tile([128, 1], BF16, name="ones_bf")
            nc.vector.memset(ones_bf[:], 1.0)
            ones_f = pp.tile([128, 1], FP32, name="ones_f")
            nc.vector.memset(ones_f[:], 1.0)
            onesrow = pp.tile([1, 32], FP32, name="onesrow")
            nc.vector.memset(onesrow[:], 1.0)
            dinv = pp.tile([128, NSB], FP32, name="dinv")
            wt = {}
            for w in ("W1", "W2", "W3"):
                wt[w] = pp.tile([F, F], FP32, name=f"t{w}")
                nc.sync.dma_start(wt[w][:], din[w][:])
            bt = {}
            for bn in ("b1r", "b2r", "b3r"):
                bt[bn] = pp.tile([128, F], FP32, name=f"t{bn}")
                nc.sync.dma_start(bt[bn][:], din[bn][:])
            gstage = pp.tile([128, NSB * TF], BF16, name="gstage")
            nc.vector.memset(gstage[:], 0.0)
            fts = [pp.tile([128, 4], FP32, name=f"ft{k}") for k in range(4)]

            gblk = dram.tile([NPBP, TF], BF16, name="gblk")
            galls = [dram.tile([NTOT, TF], BF16, name=f"gall{i}",
                               addr_space="Shared") for i in range(3)]

            # ------------- GCN helpers
            def gcn_finalize(ps, s, bias_t, relu):
                t = hp_.tile([128, F], FP32, name=f"hf{s}", tag="hfin", bufs=3)
                nc.vector.tensor_scalar(t[:], ps[:], dinv[:, s:s + 1], None,
                                        op0=OP.mult)
                nc.vector.tensor_tensor(out=t[:], in0=t[:], in1=bias_t[:],
                                        op=OP.add)
                if relu:
                    nc.vector.tensor_scalar(t[:], t[:], 0.0, None, op0=OP.max)
                return t

            def table_entry(h_tile, s, w_next):
                tp = tpp.tile([F, 128], FP32, name=f"tp{s}", tag="tp",
                              space="PSUM")
                nc.tensor.transpose(out=tp[:], in_=h_tile[:], identity=ident[:])
                fm = fmp.tile([F, 128], FP32, name=f"fmt{s}", tag="fm", bufs=2)
                nc.vector.tensor_copy(out=fm[:], in_=tp[:])
                xw = xwp.tile([128, F], FP32, name=f"xwt{s}", tag="xw",
                              space="PSUM")
                nc.tensor.matmul(xw[:], lhsT=fm[:], rhs=w_next[:], start=True,
                                 stop=True)
                nc.scalar.activation(out=gstage[:, s * TF:s * TF + F], in_=xw[:],
                                     func=AF.Copy, scale=dinv[:, s:s + 1])

            def stage_to_dram_and_ag(gl):
                out_ap = gblk[:].rearrange("(s p) f -> p s f", p=128)
                nc.gpsimd.dma_start(out_ap, gstage[:])
                nc.gpsimd.collective_compute(
                    "AllGather", OP.bypass,
                    replica_groups=[list(range(NCORES))],
                    ins=[gblk.opt()], outs=[galls[gl].opt()])

            # ------------- deg pass
            for s in range(NSB):
                nchunks = int(ncht[s, 0] + ncht[s, 1])
                ps = sps.tile([128, F], FP32, name=f"deg{s}", tag="agg",
                              space="PSUM")
                c0 = int(proc_off[s, 0])
                for j in range(nchunks):
                    oh = ohp.tile([128, 128], BF16, name=f"doh{s}_{j}", tag="oh")
                    nc.vector.tensor_scalar(oh[:], iota128[:],
                                            dstloc[:, c0 + j:c0 + j + 1], None,
                                            op0=OP.is_equal)
                    nc.tensor.matmul(ps[:, 0:1], lhsT=oh[:], rhs=ones_bf[:],
                                     start=(j == 0), stop=(j == nchunks - 1))
                dg = sp_.tile([128, 1], FP32, name=f"dg{s}", tag="dg", bufs=3)
                nc.vector.tensor_scalar(dg[:], ps[:, 0:1], 1.0, None, op0=OP.max)
                nc.scalar.activation(out=dg[:], in_=dg[:], func=AF.Sqrt)
                nc.vector.reciprocal(dinv[:, s:s + 1], dg[:])

            # ------------- g1 = dinv * (x @ W1) -> AG1
            for s in range(NSB):
                xt = sp_.tile([128, F], FP32, name=f"x{s}", tag="xld", bufs=3)
                nc.sync.dma_start(xt[:], din["xb"][s * 128:(s + 1) * 128, :])
                table_entry(xt, s, wt["W1"])
            stage_to_dram_and_ag(0)

            # ------------- ResNet stem + maxpool (overlaps AG1)
            wst = pp.tile([96, 128], BF16, name="wst")
            nc.sync.dma_start(wst[:], din["ws"][:])
            gsb = pp.tile([64, 2], FP32, name="gsb")
            nc.sync.dma_start(gsb[:, 0:1], din["gs"][:])
            nc.sync.dma_start(gsb[:, 1:2], din["bs"][:])
            x0s = []
            with tc.tile_pool(name="c1pool", bufs=1) as c1p:
                for im in range(4):
                    mpin = c1p.tile([64, 114 * 114], BF16, name=f"mpin{im}",
                                    tag="mpin", bufs=1)
                    nc.vector.memset(mpin[:], 0.0)
                    mp3 = mpin[:].rearrange("p (h w) -> p h w", w=114)
                    for c0 in range(0, 112, 28):
                        ics = [c1p.tile([96, 28 * 112], BF16,
                                        name=f"ic{im}_{c0}_{k}", tag=f"imcol{k}",
                                        bufs=2) for k in range(2)]
                        for k in range(2):
                            for tl in range(8):
                                t = k * 8 + tl
                                bb, aa = divmod(t, 4)
                                ap_in = din["xs2d"][im, :, bb + c0:bb + c0 + 28,
                                                    aa:aa + 112]
                                nc.sync.dma_start(
                                    ics[k][tl * 12:(tl + 1) * 12, :], ap_in)
                        for rr in range(0, 28, 4):
                            r0 = c0 + rr
                            ps = cpsp.tile([64, 4 * 112], FP32,
                                           name=f"c1_{im}_{r0}", tag="convps",
                                           space="PSUM")
                            for k in range(2):
                                nc.tensor.matmul(
                                    ps[:], lhsT=wst[:, k * 64:(k + 1) * 64],
                                    rhs=ics[k][:, rr * 112:(rr + 4) * 112],
                                    start=(k == 0), stop=(k == 1))
                            nc.scalar.activation(
                                out=mp3[:, 1 + r0:5 + r0, 1:113],
                                in_=ps[:].rearrange("p (h w) -> p h w", w=112),
                                func=AF.Relu, scale=gsb[:, 0:1],
                                bias=gsb[:, 1:2])
                    x0 = ap_.tile([64, 58 * 58], BF16, name=f"x0_{im}",
                                  tag="x0", bufs=4)
                    nc.vector.memset(x0[:], 0.0)
                    x03 = x0[:].rearrange("p (h w) -> p h w", w=58)
                    first = True
                    for dh in range(3):
                        for dw in range(3):
                            v = mp3[:, dh:dh + 112:2, dw:dw + 112:2]
                            if first:
                                nc.vector.tensor_copy(out=x03[:, 1:57, 1:57],
                                                      in_=v)
                                first = False
                            else:
                                nc.vector.tensor_tensor(
                                    out=x03[:, 1:57, 1:57],
                                    in0=x03[:, 1:57, 1:57], in1=v, op=OP.max)
                    x0s.append([x0])

            # ------------- GCN layer runner
            def gcn_layer(li, gall, w_next, bias_t, relu):
                res = {}
                if w_next is None:
                    res["pool"] = plp.tile([32, F], FP32, name="poolps")
                    res["cnt"] = cnp.tile([32, 1], FP32, name="cntps")
                for (s0, s1) in groups:
                    bufs = {}
                    for h in range(2):
                        ci, _ = call_of_sb[(s0, h)]
                        hh, lst, c0 = call_specs[ci]
                        nch = sum(n for _, n in lst)
                        if nch == 0:
                            continue
                        mg = mp.tile([128, nch * TF], BF16,
                                     name=f"mg{li}_{s0}_{h}", tag="msgs", bufs=2)
                        nc.gpsimd.dma_gather(
                            mg[:].rearrange("p (c f) -> p c f", f=TF),
                            gall[h * HALF:(h + 1) * HALF, :],
                            src16[:, c0 * 8:(c0 + nch) * 8],
                            nch * 128, nch * 128, TF, single_packet=False)
                        bufs[h] = mg
                    for s in range(s0, s1):
                        tot = int(ncht[s, 0] + ncht[s, 1])
                        ps = sps.tile([128, F], FP32, name=f"ag{li}_{s}",
                                      tag="agg", space="PSUM")
                        i = 0
                        for h in range(2):
                            nch = int(ncht[s, h])
                            if nch == 0:
                                continue
                            _, off = call_of_sb[(s, h)]
                            p0 = int(proc_off[s, h])
                            for j in range(nch):
                                oh = ohp.tile([128, 128], BF16,
                                              name=f"oh{li}_{s}_{h}_{j}",
                                              tag="oh")
                                nc.vector.tensor_scalar(
                                    oh[:], iota128[:],
                                    dstloc[:, p0 + j:p0 + j + 1], None,
                                    op0=OP.is_equal)
                                rhs = bufs[h][:, (off + j) * TF:(off + j) * TF + F]
                                nc.tensor.matmul(ps[:], lhsT=oh[:], rhs=rhs,
                                                 start=(i == 0),
                                                 stop=(i == tot - 1))
                                i += 1
                        ht = gcn_finalize(ps, s, bias_t, relu)
                        if w_next is not None:
                            table_entry(ht, s, w_next)
                        else:
                            bh = ohp.tile([128, 32], FP32, name=f"bh{s}",
                                          tag="bh", bufs=3)
                            nc.vector.tensor_scalar(bh[:], iota32[:],
                                                    batchv[:, s:s + 1], None,
                                                    op0=OP.is_equal)
                            nc.tensor.matmul(res["pool"][:], lhsT=bh[:],
                                             rhs=ht[:], start=(s == 0),
                                             stop=(s == NSB - 1))
                            nc.tensor.matmul(res["cnt"][:], lhsT=bh[:],
                                             rhs=ones_f[:], start=(s == 0),
                                             stop=(s == NSB - 1))
                return res

            # ------------- ResNet block machinery
            def conv_shift(wpool, name, src_aps, cin, hp_in, cout, h_out,
                           stride, taps, wsb, out_writer):
                n_k = len(src_aps)
                n_m = (cout + 127) // 128
                rows = max(1, min(504 // h_out, h_out))
                for m in range(n_m):
                    mt = min(128, cout - m * 128)
                    for r0 in range(0, h_out, rows):
                        r = min(rows, h_out - r0)
                        ps = cpsp.tile([mt, r * h_out], FP32,
                                       name=f"{name}m{m}r{r0}", tag="convps",
                                       space="PSUM")
                        i, last = 0, len(taps) * n_k - 1
                        for (dh, dw) in taps:
                            for k in range(n_k):
                                src3 = src_aps[k].rearrange(
                                    "p (h w) -> p h w", w=hp_in)
                                rhs = src3[:,
                                           dh + r0 * stride:
                                           dh + (r0 + r) * stride:stride,
                                           dw:dw + h_out * stride:stride]
                                ti = dh * 3 + dw if len(taps) == 9 else 0
                                lhsT = wsb[k][:, ti * cout + m * 128:
                                              ti * cout + m * 128 + mt]
                                nc.tensor.matmul(ps[:], lhsT=lhsT, rhs=rhs,
                                                 start=(i == 0),
                                                 stop=(i == last))
                                i += 1
                        out_writer(m, mt, r0, r, ps)

            def load_conv_w(wpool, nm, cin, cols, tag):
                tiles = []
                for k in range((cin + 127) // 128):
                    ct = min(128, cin - k * 128)
                    t = wpool.tile([ct, cols], BF16, name=f"{nm}k{k}", tag=tag,
                                   bufs=6)
                    nc.sync.dma_start(t[:], din[nm][k * 128:k * 128 + ct, :])
                    tiles.append(t)
                return tiles

            def load_bn(wpool, gn, bn, cout):
                n_m = (cout + 127) // 128
                t = wpool.tile([128, 2 * n_m], FP32, name=f"bn{gn}", tag="bn",
                               bufs=6)
                for m in range(n_m):
                    mt = min(128, cout - m * 128)
                    nc.sync.dma_start(t[:mt, 2 * m:2 * m + 1],
                                      din[gn][m * 128:m * 128 + mt, :])
                    nc.sync.dma_start(t[:mt, 2 * m + 1:2 * m + 2],
                                      din[bn][m * 128:m * 128 + mt, :])
                return t

            def res_block(wpool, bi, x_tiles, im, out_pad):
                cin, cout, h_in, stride, has_ds = _BLOCK_CFG[bi]
                h_out = h_in // stride
                hp_in, hp_out = h_in + 2, h_out + 2
                n_m = (cout + 127) // 128
                st = _STAGE_OF_BLOCK[bi]
                wa = load_conv_w(wpool, f"w{bi}a", cin, 9 * cout, f"cw{st}")
                bna = load_bn(wpool, f"g{bi}a", f"b{bi}a", cout)
                mids = []
                for m in range(n_m):
                    mt = min(128, cout - m * 128)
                    md = wpool.tile([mt, hp_out * hp_out], BF16,
                                    name=f"mid{bi}_{im}_{m}", tag=f"mid{st}",
                                    bufs=2 * n_m)
                    nc.vector.memset(md[:], 0.0)
                    mids.append(md)

                def w_mid(m, mt, r0, r, ps):
                    md3 = mids[m][:].rearrange("p (h w) -> p h w", w=hp_out)
                    nc.scalar.activation(
                        out=md3[:, 1 + r0:1 + r0 + r, 1:1 + h_out],
                        in_=ps[:].rearrange("p (h w) -> p h w", w=h_out),
                        func=AF.Relu, scale=bna[:mt, 2 * m:2 * m + 1],
                        bias=bna[:mt, 2 * m + 1:2 * m + 2])

                conv_shift(wpool, f"c{bi}a{im}", [t[:] for t in x_tiles], cin,
                           hp_in, cout, h_out, stride, TAPS9, wa, w_mid)

                wb = load_conv_w(wpool, f"w{bi}b", cout, 9 * cout, f"cw{st}")
                bnb = load_bn(wpool, f"g{bi}b", f"b{bi}b", cout)
                t2s = []
                for m in range(n_m):
                    mt = min(128, cout - m * 128)
                    t2s.append(wpool.tile([mt, h_out * h_out], BF16,
                                          name=f"t2_{bi}_{im}_{m}",
                                          tag=f"t2{st}", bufs=2 * n_m))

                def w_t2(m, mt, r0, r, ps):
                    nc.scalar.activation(
                        out=t2s[m][:, r0 * h_out:(r0 + r) * h_out], in_=ps[:],
                        func=AF.Identity, scale=bnb[:mt, 2 * m:2 * m + 1],
                        bias=bnb[:mt, 2 * m + 1:2 * m + 2])

                conv_shift(wpool, f"c{bi}b{im}", [t[:] for t in mids], cout,
                           hp_out, cout, h_out, 1, TAPS9, wb, w_t2)

                idn3s = []
                if has_ds:
                    wd = load_conv_w(wpool, f"wd{bi}", cin, cout, f"cw{st}")
                    bnd = load_bn(wpool, f"gd{bi}", f"bd{bi}", cout)
                    tis = []
                    for m in range(n_m):
                        mt = min(128, cout - m * 128)
                        tis.append(wpool.tile([mt, h_out * h_out], BF16,
                                              name=f"idn{bi}_{im}_{m}",
                                              tag=f"t2{st}", bufs=2 * n_m))

                    def w_idn(m, mt, r0, r, ps):
                        nc.scalar.activation(
                            out=tis[m][:, r0 * h_out:(r0 + r) * h_out],
                            in_=ps[:], func=AF.Identity,
                            scale=bnd[:mt, 2 * m:2 * m + 1],
                            bias=bnd[:mt, 2 * m + 1:2 * m + 2])

                    conv_shift(wpool, f"cd{bi}{im}", [t[:] for t in x_tiles],
                               cin, hp_in, cout, h_out, stride, [(1, 1)], wd,
                               w_idn)
                    idn3s = [t[:].rearrange("p (h w) -> p h w", w=h_out)
                             for t in tis]
                else:
                    idn3s = [t[:].rearrange("p (h w) -> p h w", w=hp_in)[
                        :, 1:1 + h_out, 1:1 + h_out] for t in x_tiles]

                outs = []
                for m in range(n_m):
                    mt = min(128, cout - m * 128)
                    if out_pad:
                        ot = ap_.tile([mt, hp_out * hp_out], BF16,
                                      name=f"ro{bi}_{im}_{m}", tag=f"ro{bi}",
                                      bufs=4 * n_m if bi in (1, 5) else 2 * n_m)
                        nc.vector.memset(ot[:], 0.0)
                        dst = ot[:].rearrange("p (h w) -> p h w", w=hp_out)[
                            :, 1:1 + h_out, 1:1 + h_out]
                    else:
                        ot = ap_.tile([mt, h_out * h_out], BF16,
                                      name=f"ro{bi}_{im}_{m}", tag=f"ro{bi}",
                                      bufs=2 * n_m)
                        dst = ot[:].rearrange("p (h w) -> p h w", w=h_out)
                    t23 = t2s[m][:].rearrange("p (h w) -> p h w", w=h_out)
                    nc.vector.tensor_tensor(out=t23, in0=t23, in1=idn3s[m],
                                            op=OP.add)
                    nc.vector.tensor_scalar(dst, t23, 0.0, None, op0=OP.max)
                    outs.append(ot)
                return outs

            # ------------- phase interleaving
            gcn_layer(0, galls[0], wt["W2"], bt["b1r"], True)
            stage_to_dram_and_ag(1)

            acts = x0s
            with tc.tile_pool(name="w1pool", bufs=1) as w1p:
                for im in range(4):
                    a = res_block(w1p, 0, acts[im], im, True)
                    acts[im] = res_block(w1p, 1, a, im, True)

            gcn_layer(1, galls[1], wt["W3"], bt["b2r"], True)
            stage_to_dram_and_ag(2)

            with tc.tile_pool(name="w23pool", bufs=1) as w23p:
                for im in range(4):
                    a = res_block(w23p, 2, acts[im], im, True)
                    a = res_block(w23p, 3, a, im, True)
                    a = res_block(w23p, 4, a, im, True)
                    acts[im] = res_block(w23p, 5, a, im, True)

            r3 = gcn_layer(2, galls[2], None, bt["b3r"], False)
            pc = sp_.tile([32, 65], FP32, name="pc", tag="pc", bufs=2)
            nc.vector.tensor_copy(out=pc[:, 0:64], in_=r3["pool"][:])
            nc.vector.tensor_copy(out=pc[:, 64:65], in_=r3["cnt"][:])
            tpc = tpp.tile([65, 32], FP32, name="tpc", tag="tp", space="PSUM")
            nc.tensor.transpose(out=tpc[:], in_=pc[:], identity=ident[:32, :32])
            arin = sp_.tile([65, 32], FP32, name="arin", tag="arin", bufs=2)
            nc.vector.tensor_copy(out=arin[:], in_=tpc[:])
            ar_d = dram.tile([65, 32], FP32, name="ar_d")
            nc.gpsimd.dma_start(ar_d[:], arin[:])
            ar_o = dram.tile([65, 32], FP32, name="ar_o", addr_space="Shared")
            nc.gpsimd.collective_compute(
                "AllReduce", OP.add, replica_groups=[list(range(NCORES))],
                ins=[ar_d.opt()], outs=[ar_o.opt()])

            with tc.tile_pool(name="w4pool", bufs=1) as w4p:
                for im in range(4):
                    a = res_block(w4p, 6, acts[im], im, True)
                    a = res_block(w4p, 7, a, im, False)
                    for k in range(4):
                        nc.vector.tensor_reduce(out=fts[k][:, im:im + 1],
                                                in_=a[k][:],
                                                axis=mybir.AxisListType.X,
                                                op=OP.add)
            fa_d = dram.tile([512, 4], FP32, name="fa_d")
            for k in range(4):
                nc.gpsimd.dma_start(fa_d[k * 128:(k + 1) * 128, :], fts[k][:])
            fa_o = dram.tile([NCORES * 512, 4], FP32, name="fa_o",
                             addr_space="Shared")
            nc.gpsimd.collective_compute(
                "AllGather", OP.bypass, replica_groups=[list(range(NCORES))],
                ins=[fa_d.opt()], outs=[fa_o.opt()])

            # ------------- head
            pfm = sp_.tile([65, 32], FP32, name="pfm", tag="arin", bufs=2)
            nc.sync.dma_start(pfm[:], ar_o[:])
            t1 = tpp.tile([32, 65], FP32, name="t1h", tag="tp", space="PSUM")
            nc.tensor.transpose(out=t1[:], in_=pfm[:], identity=ident[:65, :65])
            pcn = sp_.tile([32, 65], FP32, name="pcn", tag="pc", bufs=2)
            nc.vector.tensor_copy(out=pcn[:], in_=t1[:])
            rc = sp_.tile([32, 1], FP32, name="rc", tag="dg", bufs=3)
            nc.vector.tensor_scalar(rc[:], pcn[:, 64:65], 1.0, None, op0=OP.max)
            nc.vector.reciprocal(rc[:], rc[:])
            nc.vector.tensor_scalar(pcn[:, 0:64], pcn[:, 0:64], rc[:], None,
                                    op0=OP.mult)
            t2p = tpp.tile([64, 32], FP32, name="t2ph", tag="tp", space="PSUM")
            nc.tensor.transpose(out=t2p[:], in_=pcn[:, 0:64],
                                identity=ident[:32, :32])
            pfm2 = sp_.tile([64, 32], FP32, name="pfm2", tag="arin", bufs=2)
            nc.vector.tensor_copy(out=pfm2[:], in_=t2p[:])
            hps = xwp.tile([32, NCLS], FP32, name="hps", tag="xw", space="PSUM")
            lw0 = sp_.tile([64, NCLS], FP32, name="lw0", tag="lw", bufs=6)
            nc.sync.dma_start(lw0[:], din["linW"][0:64, :])
            nc.tensor.matmul(hps[:], lhsT=pfm2[:], rhs=lw0[:], start=True,
                             stop=False)
            fa_r = fa_o[:].rearrange("(c k p) i -> k p c i", k=4, p=128)
            for k in range(4):
                fl = sp_.tile([128, 32], FP32, name=f"fl{k}", tag="fl", bufs=4)
                nc.sync.dma_start(fl[:], fa_r[k])
                lwk = sp_.tile([128, NCLS], FP32, name=f"lw{k + 1}", tag="lw",
                               bufs=6)
                nc.sync.dma_start(lwk[:],
                                  din["linW"][64 + k * 128:64 + (k + 1) * 128, :])
                nc.tensor.matmul(hps[:], lhsT=fl[:], rhs=lwk[:], start=False,
                                 stop=False)
            lbt = sp_.tile([1, NCLS], FP32, name="lbt", tag="lbt", bufs=1)
            nc.sync.dma_start(lbt[:], din["linb"][:])
            nc.tensor.matmul(hps[:], lhsT=onesrow[:], rhs=lbt[:], start=False,
                             stop=True)
            outsb = sp_.tile([32, NCLS], FP32, name="outsb", tag="outsb",
                             bufs=1)
            nc.scalar.activation(out=outsb[:], in_=hps[:], func=AF.Copy)
            nc.sync.dma_start(out_d[:], outsb[:])

    nc.compile()
    return nc


# ---------------------------------------------------------------- entry point

_CACHE = {}


def _build(meta, resw):
    key = ("k", meta["NCH"])
    if key not in _CACHE:
        _CACHE[key] = build_kernel(meta, resw)
    return _CACHE[key]


def kernel(x, edge_index, x_image, batch, W1, b1, W2, b2, W3, b3, lin_W, lin_b,
           resnet_params):
    x = np.asarray(x, np.float32)
    edge_index = np.asarray(edge_index)
    x_image = np.asarray(x_image, np.float32)
    batch = np.asarray(batch)

    meta = _prep_edges(edge_index)
    resw = _prep_resnet(resnet_params)

    linW = np.asarray(lin_W, np.float32).copy()
    linW[64:, :] /= 49.0

    in_maps = []
    for c in range(NCORES):
        m = {}
        xb = np.zeros((NPBP, F), np.float32)
        xb[:NPB] = x[c * NPB:(c + 1) * NPB]
        m["xb"] = xb
        m["idx16"] = meta["idx16"][c]
        m["dstloc"] = np.ascontiguousarray(meta["dstloc"][c])
        bpad = np.full(NPBP, -1.0, np.float32)
        bpad[:NPB] = np.asarray(batch[c * NPB:(c + 1) * NPB], np.float32)
        m["batchv"] = np.ascontiguousarray(bpad.reshape(NSB, 128).T)
        m["W1"] = np.asarray(W1, np.float32)
        m["W2"] = np.asarray(W2, np.float32)
        m["W3"] = np.asarray(W3, np.float32)
        m["b1r"] = np.tile(np.asarray(b1, np.float32)[None, :], (128, 1))
        m["b2r"] = np.tile(np.asarray(b2, np.float32)[None, :], (128, 1))
        m["b3r"] = np.tile(np.asarray(b3, np.float32)[None, :], (128, 1))
        m["linW"] = linW
        m["linb"] = np.asarray(lin_b, np.float32)[None, :]
        m["xs2d"] = np.stack([_s2d_pad(x_image[c * 4 + i]) for i in range(4)])
        for k, v in resw.items():
            m[k] = v
        in_maps.append(m)

    nc = _build(meta, resw)
    res = bass_utils.run_bass_kernel_spmd(nc, in_maps,
                                          core_ids=list(range(NCORES)))
    return np.asarray(res.results[0]["out"])
